# revision 1
# baseline (speedup 1.0000x reference)
"""Trainium2 Bass kernel for nn_DiagonalTraining (anti-diagonal per-diag Linear).

out[b, r, c] = sum_{r'} W[d, r - r0(d), r' - r0(d)] * x[b, r', d - r'] + bias,
with d = r + c, over the valid range of r' for diagonal d.

Strategy: shard the 511 independent diagonals across 8 cores (expert-style).
The host packs each core's work into uniform-shape matmul jobs:
  - short diagonals (n <= 128): pair-packed into bins of K=128 (block-diag W),
    one matmul [K=128] x [N=128] per bin, 17 bins/core.
  - long diagonals (128 < n <= 256): one job each, PSUM-accumulated over 2
    K-chunks of 128, N=256 outputs, 32 jobs/core.
Stationary operand = gathered diagonal data xd^T [K, batch=128]; moving
operand = per-diagonal weights [K, N]. PSUM out = [batch=128, N].
Host scatters the packed outputs back to the grid and adds bias.
"""

import sys

sys.path.insert(0, "/opt/trn_rl_repo")

import numpy as np

B, S = 128, 256
D = 2 * S - 1  # 511
NCORES = 8
NSB = 17  # short-diagonal bins per core
NLJ = 32  # long-diagonal jobs per core

USE_BF16 = False  # flipped after precision/perf measurement
USE_F32R = True  # float32r: same fp32 bits, full-rate PE streaming at N>=256
TRACE = False  # test.py sets True to pull exec_time_ns from the NTFF profile
last_results = None


def _geom(d):
    r0 = max(0, d - S + 1)
    n = d + 1 if d < S else 2 * S - 1 - d
    return r0, n


def _job_tables():
    """Static per-core packing tables (indices + masks + scatter targets)."""
    # ---- short bins: 129 real bins + 7 dummies = 136 = 8 * 17
    sbins = []
    for kk in range(1, 64):
        sbins.append([kk - 1, 127 - kk])
        sbins.append([511 - kk, 383 + kk])
    sbins.append([63, 447])
    sbins.append([127])
    sbins.append([383])
    sbins += [[] for _ in range(136 - len(sbins))]
    # ---- long jobs: d in [128, 382] (255) + 1 dummy = 256 = 8 * 32
    ljobs = [[d] for d in range(128, 383)] + [[]]

    cores = []
    for c in range(NCORES):
        my_s = sbins[c::NCORES]
        my_l = ljobs[c::NCORES]
        xds_i = np.zeros((NSB, 128), np.int64)
        xds_m = np.zeros((NSB, 128), np.float32)
        ws_i = np.zeros((NSB, 128, 128), np.int64)
        ws_m = np.zeros((NSB, 128, 128), np.float32)
        tgt_s = np.full((NSB, 128), -1, np.int64)
        for j, bin_ds in enumerate(my_s):
            off = 0
            for d in bin_ds:
                r0, n = _geom(d)
                i = np.arange(n)
                r = r0 + i
                col = d - r
                xds_i[j, off : off + n] = r * S + col
                xds_m[j, off : off + n] = 1.0
                # W[d, m, k] at [k, m] (k = contraction pos, m = output pos)
                ws_i[j, off : off + n, off : off + n] = (
                    d * S * S + i[None, :] * S + i[:, None]
                )
                ws_m[j, off : off + n, off : off + n] = 1.0
                tgt_s[j, off : off + n] = r * S + col
                off += n

        xdl_i = np.zeros((NLJ, 2, 128), np.int64)
        xdl_m = np.zeros((NLJ, 2, 128), np.float32)
        wl_i = np.zeros((NLJ, 2, 128, 256), np.int64)
        wl_m = np.zeros((NLJ, 2, 128, 256), np.float32)
        tgt_l = np.full((NLJ, 256), -1, np.int64)
        for j, job in enumerate(my_l):
            if not job:
                continue
            (d,) = job
            r0, n = _geom(d)
            m = np.arange(256)
            for ch in range(2):
                i = ch * 128 + np.arange(128)
                v = i < n
                r = r0 + np.minimum(i, n - 1)
                xdl_i[j, ch] = (r * S + (d - r)) * v
                xdl_m[j, ch] = v.astype(np.float32)
                mv = (m < n)[None, :] & v[:, None]
                wl_i[j, ch] = (d * S * S + np.minimum(m, n - 1)[None, :] * S + np.minimum(i, n - 1)[:, None]) * mv
                wl_m[j, ch] = mv.astype(np.float32)
            mr = r0 + m[: n]
            tgt_l[j, :n] = mr * S + (d - mr)
        cores.append(
            dict(
                xds_i=xds_i, xds_m=xds_m, ws_i=ws_i, ws_m=ws_m, tgt_s=tgt_s,
                xdl_i=xdl_i, xdl_m=xdl_m, wl_i=wl_i, wl_m=wl_m, tgt_l=tgt_l,
            )
        )
    # bias gather: out_flat[p] += b[d, r - r0(d)] for p = r*S + c, d = r + c
    rr, cc = np.divmod(np.arange(S * S), S)
    dd = rr + cc
    r0v = np.maximum(0, dd - S + 1)
    bidx = dd * S + (rr - r0v)
    return cores, bidx


_TABLES = None
_PROG = {}


def _tables():
    global _TABLES
    if _TABLES is None:
        _TABLES = _job_tables()
    return _TABLES


def _build_program(use_bf16):
    import concourse.bass as bass
    import concourse.mybir as mybir
    import concourse.tile as tile

    f32 = mybir.dt.float32
    if use_bf16:
        dt_in = mybir.dt.bfloat16
    elif USE_F32R:
        dt_in = mybir.dt.float32r
    else:
        dt_in = f32
    nc = bass.Bass()
    bl = nc.dram_tensor("bl", [128, NLJ * 2 * 384], dt_in, kind="ExternalInput")
    bs = nc.dram_tensor("bs", [128, NSB * 256], dt_in, kind="ExternalInput")
    ys = nc.dram_tensor("ys", [128, NSB * 128], f32, kind="ExternalOutput")
    yl = nc.dram_tensor("yl", [128, NLJ * 256], f32, kind="ExternalOutput")

    CH = 4  # L-jobs per load group
    NPS = 6  # psum slots (full banks, cycled)
    SG_BOUNDS = [(0, 8), (8, NSB)]  # S-bin load groups

    # SBUF staging (no reuse -> no WAR deps on input DMAs)
    BTL = [
        nc.alloc_sbuf_tensor(f"btl{g}", [128, CH * 2 * 384], dt_in).ap()
        for g in range(NLJ // CH)
    ]
    BTS = [
        nc.alloc_sbuf_tensor(f"bts{g}", [128, (j1 - j0) * 256], dt_in).ap()
        for g, (j0, j1) in enumerate(SG_BOUNDS)
    ]
    YL = nc.alloc_sbuf_tensor("YL", [128, NLJ * 256], f32).ap()
    YS = nc.alloc_sbuf_tensor("YS", [128, NSB * 128], f32).ap()
    PS = [
        nc.alloc_psum_tensor(f"ps{i}", [128, 512], f32).ap() for i in range(NPS)
    ]

    # unified job list: (required_input_dma_count, n_chunks, lhs/rhs slices, out)
    jobs = []
    for j in range(NLJ):
        g = j // CH
        jj = j % CH
        ops = []
        for ch in range(2):
            o = (jj * 2 + ch) * 384
            ops.append((BTL[g], o))
        jobs.append(("L", g + 1, ops, j))
    n_l_dma = NLJ // CH
    for gi, (j0, j1) in enumerate(SG_BOUNDS):
        for j in range(j0, j1):
            o = (j - j0) * 256
            jobs.append(("S", n_l_dma + gi + 1, [(BTS[gi], o)], j))

    DIN = [
        nc.alloc_semaphore(f"din{i}")
        for i in range(NLJ // CH + len(SG_BOUNDS))
    ]  # one per input DMA (completion order across queues is not FIFO)
    P = nc.alloc_semaphore("P")  # PE job completions
    C = nc.alloc_semaphore("C")  # DVE copy completions
    DO = nc.alloc_semaphore("DO")  # output DMA completions (x16)

    with nc.Block() as block:

        @block.sync
        def _(sync):
            for g in range(n_l_dma):
                sync.dma_start(
                    out=BTL[g][:], in_=bl[:, g * CH * 2 * 384 : (g + 1) * CH * 2 * 384]
                ).then_inc(DIN[g], 16)
            for gi, (j0, j1) in enumerate(SG_BOUNDS):
                sync.dma_start(
                    out=BTS[gi][:], in_=bs[:, j0 * 256 : j1 * 256]
                ).then_inc(DIN[n_l_dma + gi], 16)
            n_out = 0
            for g in range(n_l_dma):
                sync.wait_ge(C, (g + 1) * CH)
                sync.dma_start(
                    out=yl[:, g * CH * 256 : (g + 1) * CH * 256],
                    in_=YL[:, g * CH * 256 : (g + 1) * CH * 256],
                ).then_inc(DO, 16)
                n_out += 1
            for gi, (j0, j1) in enumerate(SG_BOUNDS):
                sync.wait_ge(C, NLJ + j1)
                sync.dma_start(
                    out=ys[:, j0 * 128 : j1 * 128], in_=YS[:, j0 * 128 : j1 * 128]
                ).then_inc(DO, 16)
                n_out += 1
            sync.wait_ge(DO, 16 * n_out)

        @block.tensor
        def _(tensor):
            cur_d = 0
            for ji, (kind, dthr, ops, j) in enumerate(jobs):
                if dthr > cur_d:
                    tensor.wait_ge(DIN[dthr - 1], 16)
                    cur_d = dthr
                if ji >= NPS:
                    tensor.wait_ge(C, ji - NPS + 1)
                ps = PS[ji % NPS]
                if kind == "L":
                    for ch, (bt, o) in enumerate(ops):
                        mm = nc.tensor.matmul(
                            ps[:, 0:256],
                            bt[:, o : o + 128],
                            bt[:, o + 128 : o + 384],
                            start=(ch == 0),
                            stop=(ch == 1),
                        )
                else:
                    (bt, o) = ops[0]
                    mm = nc.tensor.matmul(
                        ps[:, 0:128],
                        bt[:, o : o + 128],
                        bt[:, o + 128 : o + 256],
                        start=True,
                        stop=True,
                    )
                mm.then_inc(P, 1)

        @block.vector
        def _(vector):
            for ji, (kind, dthr, ops, j) in enumerate(jobs):
                vector.wait_ge(P, ji + 1)
                ps = PS[ji % NPS]
                if kind == "L":
                    cp = nc.vector.tensor_copy(
                        YL[:, j * 256 : (j + 1) * 256], ps[:, 0:256]
                    )
                else:
                    cp = nc.vector.tensor_copy(
                        YS[:, j * 128 : (j + 1) * 128], ps[:, 0:128]
                    )
                cp.then_inc(C, 1)

    return nc


def _get_program(use_bf16):
    if use_bf16 not in _PROG:
        _PROG[use_bf16] = _build_program(use_bf16)
    return _PROG[use_bf16]


def _pack_core(t, x_flat, W_flat, np_dt):
    xds = (x_flat[:, t["xds_i"]] * t["xds_m"]).astype(np_dt)  # [B, NSB, 128]
    XDS = xds.transpose(2, 1, 0)  # [128k, NSB, 128b]
    ws = (W_flat[t["ws_i"]] * t["ws_m"]).astype(np_dt)  # [NSB, 128k, 128m]
    WS = ws.transpose(1, 0, 2)  # [128k, NSB, 128m]
    BS = np.concatenate([XDS, WS], axis=2).reshape(128, NSB * 256)
    xdl = (x_flat[:, t["xdl_i"]] * t["xdl_m"]).astype(np_dt)  # [B, NLJ, 2, 128]
    XDL = xdl.transpose(3, 1, 2, 0).reshape(128, NLJ * 2, 128)
    wldat = (W_flat[t["wl_i"]] * t["wl_m"]).astype(np_dt)  # [NLJ, 2, 128, 256]
    WL = wldat.transpose(2, 0, 1, 3).reshape(128, NLJ * 2, 256)
    BL = np.concatenate([XDL, WL], axis=2).reshape(128, NLJ * 2 * 384)
    return {
        "bl": np.ascontiguousarray(BL),
        "bs": np.ascontiguousarray(BS),
    }


def kernel(x, W, b):
    import ml_dtypes
    from concourse.bass_utils import run_bass_kernel_spmd

    x = np.asarray(x, np.float32)
    W = np.asarray(W, np.float32)
    b = np.asarray(b, np.float32)
    cores, bidx = _tables()
    np_dt = ml_dtypes.bfloat16 if USE_BF16 else np.float32
    x_flat = x.reshape(B, S * S)
    W_flat = W.reshape(-1)
    in_maps = [_pack_core(t, x_flat, W_flat, np_dt) for t in cores]
    nc = _get_program(USE_BF16)
    res = run_bass_kernel_spmd(
        nc, in_maps, core_ids=list(range(NCORES)), trace=TRACE
    )
    global last_results
    last_results = res
    out_flat = np.zeros((B, S * S), np.float32)
    for c, t in enumerate(cores):
        ysv = res.results[c]["ys"].reshape(B, -1)
        ylv = res.results[c]["yl"].reshape(B, -1)
        fs = t["tgt_s"].reshape(-1)
        vs = fs >= 0
        out_flat[:, fs[vs]] = ysv[:, vs]
        fl = t["tgt_l"].reshape(-1)
        vl = fl >= 0
        out_flat[:, fl[vl]] = ylv[:, vl]
    out_flat += b.reshape(-1)[bidx][None, :]
    return out_flat.reshape(B, S, S)



# revision 12
# speedup vs baseline: 2.0418x; 2.0418x over previous
"""Trainium2 Bass kernel for nn_DiagonalTraining (anti-diagonal per-diag Linear).

out[b, r, c] = sum_{r'} W[d, r - r0(d), r' - r0(d)] * x[b, r', d - r'] + bias,
with d = r + c over the valid range of r' for diagonal d.

v2 strategy (v1 was f32 + heavy padding, 20.1 MB DMA/core, 71.7 us, fully
DMA-bound): cut DMA bytes ~2.7x.

- All device traffic is bf16 (inputs, weights, outputs; PSUM accumulates f32).
  Measured numpy rel-err 2.9e-3 vs the 2e-2 gate.
- Diagonal lengths are padded up to multiples of 4. Each length-class
  (n' = 4j, j=1..64) contains exactly 8 diagonals (class 256 gets 1 dummy),
  so dealing one member per core gives every core the SAME shape schedule:
  required for the single SPMD program.
- Per core: 32 "pairs", each = one long diag (n' = 132..256, two K-chunks
  PSUM-accumulated) + one short diag (n' = 128..4, one chunk), sharing a
  260-col PSUM region in bank s%8.
- K-chunks sit at 32-aligned SBUF partition offsets (PE quadrant grid rule:
  K<=32 at 0/32/64/96, K<=64 at 0/64, else 0). The host bin-packs the
  xd [K,128] and W [K,n'] tiles of partial chunks into shared 128-partition
  rectangles, so the flat group DMAs move almost no padding.
- PSUM->SBUF copies (f32->bf16) and the output DMAs are split across
  vector/scalar/gpsimd; the 8 input group DMAs are issued by sync.
"""

import sys

sys.path.insert(0, "/opt/trn_rl_repo")

import numpy as np

B, S = 128, 256
D = 2 * S - 1  # 511
NCORES = 8
GRAN = 4
NCLS = 64
NPAIRS = 32
NGROUPS = 8
PAIR_W = 260  # n'L + n'S, constant across pairs
YCOLS = NPAIRS * PAIR_W  # 8320

USE_BF16 = True  # kept for test.py compat; v2 is always bf16
TRACE = False
last_results = None


def _geom(d):
    r0 = max(0, d - S + 1)
    n = d + 1 if d < S else 2 * S - 1 - d
    return r0, n


def _ceil32(k):
    return ((k + 31) // 32) * 32


def _classes():
    """class j (1..64): diagonals with n in (4j-4, 4j]; each has 8 members
    (class 64: 7 real + 1 dummy None)."""
    cls = [[] for _ in range(NCLS + 1)]
    for d in range(D):
        _, n = _geom(d)
        cls[(n + GRAN - 1) // GRAN].append(d)
    cls[NCLS].append(None)
    for j in range(1, NCLS + 1):
        assert len(cls[j]) == 8, (j, len(cls[j]))
    return cls


def _build_layout():
    """Shape-level schedule + column layout, identical for all cores.

    Returns dict:
      pairs[s] = (jL, jS)
      chunks[s] = list of (tag, k0, K, w, pbase, xcol, wcol)
                  tag in {L1, L2, SS}; AP partition base pbase; xd tile at
                  IN[pbase:pbase+K, xcol:xcol+128]; W tile at
                  IN[pbase:pbase+K, wcol:wcol+w].
      groups = [(c0, c1)] * 8  input column ranges
      Lin
    """
    pairs = [(33 + i, 32 - i) for i in range(NPAIRS)]  # ascending W bytes
    chunks = [None] * NPAIRS
    groups = []
    cur = 0
    for g in range(NGROUPS):
        c0 = cur
        full = []  # (s, tag, k0, w)
        part = []  # (s, tag, k0, K, w)
        for t in range(4):
            s = g * 4 + t
            jL, jS = pairs[s]
            nL, nS = GRAN * jL, GRAN * jS
            full.append((s, "L1", 0, nL))
            part.append((s, "L2", 128, nL - 128, nL))
            part.append((s, "SS", 0, nS, nS))
            chunks[s] = []
        placed = {}  # (s, tag) -> (k0, K, w, pbase, xcol, wcol)
        for s, tag, k0, w in full:
            xcol = cur
            cur += 128
            wcol = cur
            cur += w
            placed[(s, tag)] = (k0, 128, w, 0, xcol, wcol)
        # first-fit-decreasing bin pack of partial chunks onto 4 strips.
        # AP base partition must be 0/32/64 (quadrant grid, 96 rejected by
        # bass), so strip 3 is only reachable as the tail of a >=2-strip
        # placement.
        bins = []  # dict(free=[bool]*4, wmax, members=[(s,tag,k0,K,w,pbase)])
        BASES = {4: [0], 3: [0], 2: [0, 2], 1: [0, 1, 2]}
        for s, tag, k0, K, w in sorted(
            part, key=lambda it: (-_ceil32(it[3]), -it[4])
        ):
            ns = _ceil32(K) // 32
            done = False
            for bn in bins:
                for ba in BASES[ns]:
                    if all(bn["free"][ba : ba + ns]):
                        for q in range(ba, ba + ns):
                            bn["free"][q] = False
                        bn["wmax"] = max(bn["wmax"], w)
                        bn["members"].append((s, tag, k0, K, w, ba * 32))
                        done = True
                        break
                if done:
                    break
            if not done:
                bn = dict(free=[True] * 4, wmax=w, members=[])
                for q in range(ns):
                    bn["free"][q] = False
                bn["members"].append((s, tag, k0, K, w, 0))
                bins.append(bn)
        for bn in bins:
            xcol = cur
            cur += 128
            wcol = cur
            cur += bn["wmax"]
            for s, tag, k0, K, w, pbase in bn["members"]:
                placed[(s, tag)] = (k0, K, w, pbase, xcol, wcol)
        for t in range(4):
            s = g * 4 + t
            for tag in ("L1", "L2", "SS"):
                k0, K, w, pbase, xcol, wcol = placed[(s, tag)]
                chunks[s].append((tag, k0, K, w, pbase, xcol, wcol))
        groups.append((c0, cur))
    return dict(pairs=pairs, chunks=chunks, groups=groups, Lin=cur)


# copy/output-DMA blocks: block bk = pairs 4bk..4bk+3. GPSIMD cannot access
# PSUM, so copies alternate vector/scalar; gpsimd issues the vector blocks'
# output DMAs (DVE cannot issue DMAs).
BLOCK_ENG = ["vector", "scalar", "vector", "scalar", "vector", "scalar",
             "vector", "scalar"]


def _eng_pairs(eng):
    out = []
    for bk in range(NGROUPS):
        if BLOCK_ENG[bk] == eng:
            out.extend(range(4 * bk, 4 * bk + 4))
    return out


_TABLES = None
_PROG = None


def _tables():
    global _TABLES
    if _TABLES is None:
        layout = _build_layout()
        cls = _classes()
        # per-core diag assignment + scatter targets
        cores = []
        for c in range(NCORES):
            jobs = []  # per pair: (dL, dS)
            tgt = np.full(YCOLS, -1, np.int64)
            for s, (jL, jS) in enumerate(layout["pairs"]):
                dL = cls[jL][c]
                dS = cls[jS][c]
                jobs.append((dL, dS))
                y0 = s * PAIR_W
                for d, off, wpad in ((dL, 0, GRAN * jL), (dS, GRAN * jL, GRAN * jS)):
                    if d is None:
                        continue
                    r0, n = _geom(d)
                    m = np.arange(n)
                    tgt[y0 + off + m] = (r0 + m) * S + (d - r0 - m)
            cores.append(dict(jobs=jobs, tgt=tgt))
        # bias gather: out_flat[p] += b[d, r - r0(d)], p = r*S+c, d = r+c
        rr, cc = np.divmod(np.arange(S * S), S)
        dd = rr + cc
        r0v = np.maximum(0, dd - S + 1)
        bidx = dd * S + (rr - r0v)
        _TABLES = (layout, cores, bidx)
    return _TABLES


def _build_program():
    import concourse.bass as bass
    import concourse.mybir as mybir

    layout, cores, _ = _tables()
    Lin = layout["Lin"]
    f32 = mybir.dt.float32
    bf16 = mybir.dt.bfloat16

    nc = bass.Bass()
    bi = nc.dram_tensor("bi", [128, Lin], bf16, kind="ExternalInput")
    y = nc.dram_tensor("y", [128, YCOLS], bf16, kind="ExternalOutput")

    IN = nc.alloc_sbuf_tensor("IN", [128, Lin], bf16).ap()
    Y = nc.alloc_sbuf_tensor("Y", [128, YCOLS], bf16).ap()
    PS = [nc.alloc_psum_tensor(f"ps{i}", [128, 512], f32).ap() for i in range(8)]

    DIN = [nc.alloc_semaphore(f"din{g}") for g in range(NGROUPS)]
    P = nc.alloc_semaphore("P")
    CC = {e: nc.alloc_semaphore(f"C{e[0]}") for e in ("vector", "scalar")}
    DO = nc.alloc_semaphore("DO")

    eng_pairs = {e: _eng_pairs(e) for e in ("vector", "scalar")}

    def pair_region(s):
        jL, jS = layout["pairs"][s]
        return GRAN * jL, GRAN * jS  # widths (long, short)

    with nc.Block() as block:

        @block.sync
        def _(sync):
            for g, (c0, c1) in enumerate(layout["groups"]):
                sync.dma_start(out=IN[:, c0:c1], in_=bi[:, c0:c1]).then_inc(
                    DIN[g], 16
                )
            # DVE cannot issue DMAs; sync drains the vector blocks' outputs
            # (SP HWDGE, known-good 16-inc completion semantics).
            nv = 0
            for bk in range(NGROUPS):
                if BLOCK_ENG[bk] != "vector":
                    continue
                nv += 4
                sync.wait_ge(CC["vector"], nv)
                sync.dma_start(
                    out=y[:, bk * 4 * PAIR_W : (bk + 1) * 4 * PAIR_W],
                    in_=Y[:, bk * 4 * PAIR_W : (bk + 1) * 4 * PAIR_W],
                ).then_inc(DO, 16)
            sync.wait_ge(DO, 16 * NGROUPS)

        @block.tensor
        def _(tensor):
            for s in range(NPAIRS):
                if s % 4 == 0:
                    tensor.wait_ge(DIN[s // 4], 16)
                if s >= 8:
                    u = s - 8
                    e = BLOCK_ENG[u // 4]
                    thr = eng_pairs[e].index(u) + 1
                    tensor.wait_ge(CC[e], thr)
                nL, nS = pair_region(s)
                ps = PS[s % 8]
                mm = None
                for tag, k0, K, w, pbase, xcol, wcol in layout["chunks"][s]:
                    lhsT = IN[pbase : pbase + K, xcol : xcol + 128]
                    rhs = IN[pbase : pbase + K, wcol : wcol + w]
                    if tag == "L1":
                        out = ps[:, 0:nL]
                        st, sp = True, False
                    elif tag == "L2":
                        out = ps[:, 0:nL]
                        st, sp = False, True
                    else:
                        out = ps[:, nL : nL + nS]
                        st, sp = True, True
                    mm = nc.tensor.matmul(out, lhsT, rhs, start=st, stop=sp)
                mm.then_inc(P, 1)

        def copy_body(eng_name, eng_ops):
            copier = (
                nc.scalar.copy if eng_name == "scalar" else nc.vector.tensor_copy
            )
            for bk in range(NGROUPS):
                if BLOCK_ENG[bk] != eng_name:
                    continue
                for t in range(4):
                    s = 4 * bk + t
                    eng_ops.wait_ge(P, s + 1)
                    cp = copier(
                        Y[:, s * PAIR_W : (s + 1) * PAIR_W],
                        PS[s % 8][:, 0:PAIR_W],
                    )
                    cp.then_inc(CC[eng_name], 1)
                if eng_name != "vector":  # DVE cannot issue DMAs
                    eng_ops.dma_start(
                        out=y[:, bk * 4 * PAIR_W : (bk + 1) * 4 * PAIR_W],
                        in_=Y[:, bk * 4 * PAIR_W : (bk + 1) * 4 * PAIR_W],
                    ).then_inc(DO, 16)

        @block.vector
        def _(vector):
            copy_body("vector", vector)

        @block.scalar
        def _(scalar):
            copy_body("scalar", scalar)



    return nc


def _get_program():
    global _PROG
    if _PROG is None:
        _PROG = _build_program()
    return _PROG


def _pack_core(core, layout, x, W, np_bf16):
    IN = np.zeros((128, layout["Lin"]), np_bf16)
    for s, (dL, dS) in enumerate(core["jobs"]):
        for tag, k0, K, w, pbase, xcol, wcol in layout["chunks"][s]:
            d = dS if tag == "SS" else dL
            if d is None:
                continue
            r0, n = _geom(d)
            kn = min(n - k0, K)
            if kn > 0:
                kv = np.arange(k0, k0 + kn)
                IN[pbase : pbase + kn, xcol : xcol + 128] = x[
                    :, r0 + kv, d - r0 - kv
                ].T
            # rhs[k, m] = W[d, m, k0+k]; W is zero beyond [n, n] so padding
            # contributes exactly zero.
            IN[pbase : pbase + K, wcol : wcol + w] = W[d, 0:w, k0 : k0 + K].T
    return {"bi": IN}


def kernel(x, W, b):
    import ml_dtypes
    from concourse.bass_utils import run_bass_kernel_spmd

    x = np.asarray(x, np.float32)
    W = np.asarray(W, np.float32)
    b = np.asarray(b, np.float32)
    layout, cores, bidx = _tables()
    in_maps = [
        _pack_core(core, layout, x, W, ml_dtypes.bfloat16) for core in cores
    ]
    nc = _get_program()
    res = run_bass_kernel_spmd(
        nc, in_maps, core_ids=list(range(NCORES)), trace=TRACE
    )
    global last_results
    last_results = res
    out_flat = np.zeros((B, S * S), np.float32)
    for c, core in enumerate(cores):
        Yc = np.asarray(res.results[c]["y"]).astype(np.float32)
        tgt = core["tgt"]
        v = tgt >= 0
        out_flat[:, tgt[v]] = Yc[:, v]
    out_flat += b.reshape(-1)[bidx][None, :]
    return out_flat.reshape(B, S, S)
